# revision 17
# baseline (speedup 1.0000x reference)
"""Trainium2 Bass kernel for per-image masked-softmax entropy (EntropyLoss).

Math (per (n, c) segment, over the HW=512*512 elements x of heatmap[n, c]):
    mask  = x > 0
    softmax over the masked elements, entropy in bits, summed over c and
    divided by the total positive count of image n.

The entropy of a masked softmax is invariant to the stabilizing shift m, so
we may use m = 0 (randn inputs keep exp(x) <= ~e^6, no overflow):
    S_c   = sum_{x>0} exp(x)
    U_c   = sum_{x>0} x * exp(x)
    ent_c = (log S_c - U_c / S_c) / ln2          [bits]
    out_n = sum_c ent_c / sum_c count_c

Device work per segment item [128, width] (bf16 x, cast during SWDGE DMA):
    r  = relu(x)                 (DVE tensor_scalar, 4x bf16)
    a  = exp(r)                  (ACT, fused accum -> S'_c = S_c + #nonpos)
    w  = a * r                   (DVE tensor_tensor, 2x bf16)
    mk = x > 0                   (DVE tensor_scalar, 4x bf16)
    U_c, count_c                 (PE: one-hot stationary weights route each
                                  segment's column sums into PSUM row c of a
                                  single [20, 512] accumulator; one final
                                  tensor_reduce folds 512 -> 1 for all rows)
S_c is recovered on the host as S'_c - (HW - count_c) since exp(0) = 1 for
every non-positive element. Final log/divide runs on host in float64.

Tiles are allocated ONCE and round-robined manually: every pool.tile() call
creates a distinct tile object and the TileContext teardown tail scales with
object/semaphore count (measured ~1.6us shorter with 44 objects vs 114).

Schedule: half-width items for the first and last TAPER segments (fast
pipeline fill and short drain), full segments in the middle.
"""

import os

import numpy as np

N, C, H, W = 8, 20, 512, 512
HW = H * W
P = 128
F = HW // P  # 2048
NCORES = 8
LN2 = 0.6931471805599453

DATA_BUFS = int(os.environ.get("ENTROPY_DATA_BUFS", "8"))
WARM_MM = int(os.environ.get("ENTROPY_WARM_MM", "24"))
_CACHE = {}

# Head micro-taper: small first items so the first DMA lands (and the ACT
# stream starts) as early as possible; everything after runs full width.
HEAD_WIDTHS = {0: [F // 2, F // 2], 1: [F // 2, F // 2]}


def _schedule():
    """Work items [(c, lo, width, scol)] + extras [(scol, c)] for the
    spill accumulator columns of split segments."""
    items = []
    extras = []
    scol_next = C
    for c in range(C):
        widths = HEAD_WIDTHS.get(c, [F])
        lo = 0
        for i, wd in enumerate(widths):
            if i == 0:
                scol = c
            else:
                scol = scol_next
                extras.append((scol, c))
                scol_next += 1
            items.append((c, lo, wd, scol))
            lo += wd
    return items, extras, scol_next


ITEMS, EXTRAS, SCOLS = _schedule()
# Rotate c0's second half to the end: the head-split's spare half doubles as
# the tail taper, so the final exp->mult->matmul->reduce chain runs at half
# width without any extra ACT accumulator columns.
ITEMS = [ITEMS[0]] + ITEMS[2:] + [ITEMS[1]]


def _build_program():
    import concourse.bacc as bacc
    import concourse.mybir as mybir
    import concourse.tile as tile

    dt = mybir.dt
    Alu = mybir.AluOpType
    Act = mybir.ActivationFunctionType

    nc = bacc.Bacc(None, target_bir_lowering=False, debug=False)

    x_dram = nc.dram_tensor("x", [C, P, F], dt.float32, kind="ExternalInput")
    s_dram = nc.dram_tensor("s_out", [P, SCOLS], dt.float32, kind="ExternalOutput")
    r_dram = nc.dram_tensor("red_out", [C, 2], dt.float32, kind="ExternalOutput")

    items = ITEMS
    nmm = sum(w // 512 for _, _, w, _ in items)

    with tile.TileContext(nc) as tc:
        with (
            tc.tile_pool(name="const", bufs=1) as constp,
            tc.tile_pool(name="res", bufs=1) as resp,
            tc.tile_pool(name="data", bufs=1) as datap,
            tc.tile_pool(name="psum", bufs=1, space="PSUM") as psump,
        ):
            s_res = resp.tile([P, SCOLS], dt.float32)
            red = resp.tile([C, 2], dt.float32)

            u_psum = psump.tile([C, 512], dt.float32)
            c_psum = psump.tile([C, 512], dt.float32)

            # Long-lived round-robin buffers (see module docstring).
            nb = DATA_BUFS
            def mktiles(pfx):
                return [
                    datap.tile(
                        [P, F], dt.bfloat16, tag=f"{pfx}{i}", name=f"{pfx}{i}"
                    )
                    for i in range(nb)
                ]

            xs = mktiles("x")
            rs = mktiles("r")
            as_ = mktiles("a")
            ws = mktiles("w")
            mks = mktiles("m")

            # Sliding-window one-hot weights: oh[:, 20 - c : 40 - c] is a
            # [128, 20] matrix whose only nonzero column (all ones) is c.
            oh = constp.tile([P, 2 * C], dt.bfloat16)
            nc.gpsimd.memset(oh[:], 0.0)
            nc.gpsimd.memset(oh[:, C : C + 1], 1.0)

            # ACT warmup: a 1-element Exp pulls the ~1.3us activation
            # table load into the fill phase, off the first real item.
            wx = constp.tile([P, 1], dt.bfloat16)
            wa = constp.tile([P, 1], dt.bfloat16)
            nc.gpsimd.memset(wx[:], 0.0)
            nc.scalar.activation(wa[:], wx[:], Act.Exp)

            # PE warmup: dummy matmuls during the DMA fill phase keep the
            # PE busy so HAM upclocks it to 2.4 GHz before real work lands.
            if WARM_MM:
                warm = constp.tile([P, 512], dt.bfloat16)
                nc.gpsimd.memset(warm[:], 0.0)
                w_psum = psump.tile([C, 512], dt.float32)
                for i in range(WARM_MM):
                    nc.tensor.matmul(
                        w_psum[:], oh[:, 0:C], warm[:],
                        start=(i == 0), stop=(i == WARM_MM - 1),
                    )

            # The mult (and its u_psum matmuls) for item k is emitted during
            # item k+1: DVE's queue is strict FIFO and the mult waits on
            # ACT's exp, so issuing it one item late guarantees its input is
            # ready by issue time and DVE never stalls behind it.
            umm = 0  # u-stream chunk-matmul index, for start/stop flags
            cmm = 0  # c-stream chunk-matmul index

            def emit_u(prev):
                nonlocal umm
                c, width, scol, b = prev
                r_t = rs[b][:, :width]
                a_t = as_[b][:, :width]
                w_t = ws[b][:, :width]
                nc.vector.tensor_tensor(w_t, a_t, r_t, Alu.mult)
                lhsT = oh[:, C - c : 2 * C - c]
                for j in range(width // 512):
                    sl = slice(j * 512, (j + 1) * 512)
                    nc.tensor.matmul(
                        u_psum[:], lhsT, w_t[:, sl],
                        start=(umm == 0), stop=(umm == nmm - 1),
                    )
                    umm += 1

            prev = None
            for idx, (c, lo, width, scol) in enumerate(items):
                b = idx % nb
                x_t = xs[b][:, :width]
                r_t = rs[b][:, :width]
                a_t = as_[b][:, :width]
                mk_t = mks[b][:, :width]

                # SWDGE DMA casts fp32 -> bf16 on the fly.
                nc.gpsimd.dma_start(x_t, x_dram[c, :, lo : lo + width])

                nc.vector.tensor_scalar(r_t, x_t, 0.0, None, Alu.max)
                nc.scalar.activation(
                    a_t, r_t, Act.Exp,
                    accum_out=s_res[:, scol : scol + 1],
                )
                nc.vector.tensor_scalar(mk_t, x_t, 0.0, None, Alu.is_gt)

                lhsT = oh[:, C - c : 2 * C - c]
                for j in range(width // 512):
                    sl = slice(j * 512, (j + 1) * 512)
                    nc.tensor.matmul(
                        c_psum[:], lhsT, mk_t[:, sl],
                        start=(cmm == 0), stop=(cmm == nmm - 1),
                    )
                    cmm += 1

                if prev is not None:
                    emit_u(prev)
                prev = (c, width, scol, b)

            # c-stream is complete here; fold it before the trailing u-chain
            # so only the u reduce sits on the drain tail.
            nc.vector.tensor_reduce(
                red[:, 1:2], c_psum[:], mybir.AxisListType.X, Alu.add
            )
            emit_u(prev)
            nc.vector.tensor_reduce(
                red[:, 0:1], u_psum[:], mybir.AxisListType.X, Alu.add
            )
            nc.sync.dma_start(s_dram[:], s_res[:])
            nc.sync.dma_start(r_dram[:], red[:])

    nc.compile()
    return nc


def _get_program():
    if "nc" not in _CACHE:
        _CACHE["nc"] = _build_program()
    return _CACHE["nc"]


def _run(heatmap: np.ndarray, trace: bool = False):
    from concourse.bass_utils import run_bass_kernel_spmd

    nc = _get_program()
    in_maps = [
        {"x": np.ascontiguousarray(heatmap[i].reshape(C, P, F), dtype=np.float32)}
        for i in range(NCORES)
    ]
    return run_bass_kernel_spmd(nc, in_maps, list(range(NCORES)), trace=trace)


def _finalize(results) -> np.ndarray:
    """Host epilogue: a few scalars per core -> entropy[n] in float64."""
    out = np.zeros(N, dtype=np.float64)
    for n in range(NCORES):
        r = results[n]
        s_full = r["s_out"].astype(np.float64).sum(axis=0)   # [SCOLS]
        s_prime = s_full[:C].copy()
        for scol, c in EXTRAS:
            s_prime[c] += s_full[scol]
        red = r["red_out"].astype(np.float64)                # [C, 2]
        u = red[:, 0]
        cnt = red[:, 1]
        s = s_prime - (HW - cnt)                             # masked sum exp
        ent = np.zeros(C, dtype=np.float64)
        ok = s > 0
        ent[ok] = (np.log(s[ok]) - u[ok] / s[ok]) / LN2
        out[n] = ent.sum() / cnt.sum()
    return out.astype(np.float32)


def kernel(heatmap: np.ndarray) -> np.ndarray:
    heatmap = np.asarray(heatmap, dtype=np.float32)
    assert heatmap.shape == (N, C, H, W), heatmap.shape
    res = _run(heatmap, trace=False)
    return _finalize(res.results)


# revision 18
# speedup vs baseline: 1.0635x; 1.0635x over previous
"""Trainium2 Bass kernel for per-image masked-softmax entropy (EntropyLoss).

Math (per (n, c) segment, over the HW=512*512 elements x of heatmap[n, c]):
    mask  = x > 0
    softmax over the masked elements, entropy in bits, summed over c and
    divided by the total positive count of image n.

The entropy of a masked softmax is invariant to the stabilizing shift m, so
we may use m = 0 (randn inputs keep exp(x) <= ~e^6, no overflow):
    S_c   = sum_{x>0} exp(x)
    U_c   = sum_{x>0} x * exp(x)
    ent_c = (log S_c - U_c / S_c) / ln2          [bits]
    out_n = sum_c ent_c / sum_c count_c

Device work per segment item [128, width] (bf16 x, cast during SWDGE DMA):
    r  = relu(x)                 (DVE tensor_scalar, 4x bf16)
    a  = exp(r)                  (ACT, fused accum -> S'_c = S_c + #nonpos)
    w  = a * r                   (DVE tensor_tensor, 2x bf16)
    mk = x > 0                   (DVE tensor_scalar, 4x bf16)
    U_c, count_c                 (PE: one-hot stationary weights route each
                                  segment's column sums into PSUM row c of a
                                  single [20, 512] accumulator; one final
                                  tensor_reduce folds 512 -> 1 for all rows)
S_c is recovered on the host as S'_c - (HW - count_c) since exp(0) = 1 for
every non-positive element. Final log/divide runs on host in float64.

Tiles are allocated ONCE and round-robined manually: every pool.tile() call
creates a distinct tile object and the TileContext teardown tail scales with
object/semaphore count (measured ~1.6us shorter with 44 objects vs 114).

Schedule: half-width items for the first and last TAPER segments (fast
pipeline fill and short drain), full segments in the middle.
"""

import os

import numpy as np

N, C, H, W = 8, 20, 512, 512
HW = H * W
P = 128
F = HW // P  # 2048
NCORES = 8
LN2 = 0.6931471805599453

DATA_BUFS = int(os.environ.get("ENTROPY_DATA_BUFS", "4"))
WARM_MM = int(os.environ.get("ENTROPY_WARM_MM", "24"))
_CACHE = {}

# Head micro-taper: small first items so the first DMA lands (and the ACT
# stream starts) as early as possible; everything after runs full width.
HEAD_WIDTHS = {0: [F // 2, F // 2], 1: [F // 2, F // 2]}


def _schedule():
    """Work items [(c, lo, width, scol)] + extras [(scol, c)] for the
    spill accumulator columns of split segments."""
    items = []
    extras = []
    scol_next = C
    for c in range(C):
        widths = HEAD_WIDTHS.get(c, [F])
        lo = 0
        for i, wd in enumerate(widths):
            if i == 0:
                scol = c
            else:
                scol = scol_next
                extras.append((scol, c))
                scol_next += 1
            items.append((c, lo, wd, scol))
            lo += wd
    return items, extras, scol_next


ITEMS, EXTRAS, SCOLS = _schedule()
# Rotate c0's second half to the end: the head-split's spare half doubles as
# the tail taper, so the final exp->mult->matmul->reduce chain runs at half
# width without any extra ACT accumulator columns.
ITEMS = [ITEMS[0]] + ITEMS[2:] + [ITEMS[1]]


def _build_program():
    import concourse.bacc as bacc
    import concourse.mybir as mybir
    import concourse.tile as tile

    dt = mybir.dt
    Alu = mybir.AluOpType
    Act = mybir.ActivationFunctionType

    nc = bacc.Bacc(None, target_bir_lowering=False, debug=False)

    x_dram = nc.dram_tensor("x", [C, P, F], dt.float32, kind="ExternalInput")
    s_dram = nc.dram_tensor("s_out", [P, SCOLS], dt.float32, kind="ExternalOutput")
    r_dram = nc.dram_tensor("red_out", [C, 2], dt.float32, kind="ExternalOutput")

    items = ITEMS
    nmm = sum(w // 512 for _, _, w, _ in items)

    with tile.TileContext(nc) as tc:
        with (
            tc.tile_pool(name="const", bufs=1) as constp,
            tc.tile_pool(name="res", bufs=1) as resp,
            tc.tile_pool(name="data", bufs=1) as datap,
            tc.tile_pool(name="psum", bufs=1, space="PSUM") as psump,
        ):
            s_res = resp.tile([P, SCOLS], dt.float32)
            red = resp.tile([C, 2], dt.float32)

            u_psum = psump.tile([C, 512], dt.float32)
            c_psum = psump.tile([C, 512], dt.float32)

            # Long-lived round-robin buffers (see module docstring).
            nb = DATA_BUFS
            def mktiles(pfx):
                return [
                    datap.tile(
                        [P, 2 * F], dt.bfloat16, tag=f"{pfx}{i}", name=f"{pfx}{i}"
                    )
                    for i in range(nb)
                ]

            xs = mktiles("x")
            rs = mktiles("r")
            as_ = mktiles("a")
            ws = mktiles("w")
            mks = mktiles("m")

            # Sliding-window one-hot weights: oh[:, 20 - c : 40 - c] is a
            # [128, 20] matrix whose only nonzero column (all ones) is c.
            oh = constp.tile([P, 2 * C], dt.bfloat16)
            nc.gpsimd.memset(oh[:], 0.0)
            nc.gpsimd.memset(oh[:, C : C + 1], 1.0)

            # ACT warmup: a 1-element Exp pulls the ~1.3us activation
            # table load into the fill phase, off the first real item.
            wx = constp.tile([P, 1], dt.bfloat16)
            wa = constp.tile([P, 1], dt.bfloat16)
            nc.gpsimd.memset(wx[:], 0.0)
            nc.scalar.activation(wa[:], wx[:], Act.Exp)

            # PE warmup: dummy matmuls during the DMA fill phase keep the
            # PE busy so HAM upclocks it to 2.4 GHz before real work lands.
            if WARM_MM:
                warm = constp.tile([P, 512], dt.bfloat16)
                nc.gpsimd.memset(warm[:], 0.0)
                w_psum = psump.tile([C, 512], dt.float32)
                for i in range(WARM_MM):
                    nc.tensor.matmul(
                        w_psum[:], oh[:, 0:C], warm[:],
                        start=(i == 0), stop=(i == WARM_MM - 1),
                    )

            # Group tiling: 1-2 consecutive items share one tile set, with
            # per-item DMAs landing in adjacent halves. relu and ACT stay
            # per item (the fill-critical path waits only its own DMA);
            # is_gt and the mult run per GROUP - both have slack (the mask
            # consumer lags; the mult for group g is emitted during group
            # g+1), so the wider ops amortize decode + semaphore cost on
            # the critical DVE stream for free.
            groups = [[items[0]]]
            k = 1
            while k + 1 < len(items):
                groups.append([items[k], items[k + 1]])
                k += 2
            if k < len(items):
                groups.append([items[k]])

            umm = 0  # u-stream chunk-matmul index, for start/stop flags
            cmm = 0  # c-stream chunk-matmul index

            def emit_u(prev):
                nonlocal umm
                gi, subs, gw = prev
                nc.vector.tensor_tensor(
                    ws[gi][:, :gw], as_[gi][:, :gw], rs[gi][:, :gw], Alu.mult
                )
                for c, lo, width, scol, off in subs:
                    lhsT = oh[:, C - c : 2 * C - c]
                    for j in range(width // 512):
                        sl = slice(off + j * 512, off + (j + 1) * 512)
                        nc.tensor.matmul(
                            u_psum[:], lhsT, ws[gi][:, sl],
                            start=(umm == 0), stop=(umm == nmm - 1),
                        )
                        umm += 1

            prev = None
            for gi_raw, group in enumerate(groups):
                gi = gi_raw % nb
                off = 0
                subs = []
                for c, lo, width, scol in group:
                    subs.append((c, lo, width, scol, off))
                    off += width
                gw = off

                for c, lo, width, scol, soff in subs:
                    sl = slice(soff, soff + width)
                    nc.gpsimd.dma_start(
                        xs[gi][:, sl], x_dram[c, :, lo : lo + width]
                    )
                    nc.vector.tensor_scalar(
                        rs[gi][:, sl], xs[gi][:, sl], 0.0, None, Alu.max
                    )
                    nc.scalar.activation(
                        as_[gi][:, sl], rs[gi][:, sl], Act.Exp,
                        accum_out=s_res[:, scol : scol + 1],
                    )

                nc.vector.tensor_scalar(
                    mks[gi][:, :gw], xs[gi][:, :gw], 0.0, None, Alu.is_gt
                )
                for c, lo, width, scol, soff in subs:
                    lhsT = oh[:, C - c : 2 * C - c]
                    for j in range(width // 512):
                        sl = slice(soff + j * 512, soff + (j + 1) * 512)
                        nc.tensor.matmul(
                            c_psum[:], lhsT, mks[gi][:, sl],
                            start=(cmm == 0), stop=(cmm == nmm - 1),
                        )
                        cmm += 1

                if prev is not None:
                    emit_u(prev)
                prev = (gi, subs, gw)

            # c-stream is complete here; fold it before the trailing u-chain
            # so only the u reduce sits on the drain tail.
            nc.vector.tensor_reduce(
                red[:, 1:2], c_psum[:], mybir.AxisListType.X, Alu.add
            )
            emit_u(prev)
            nc.vector.tensor_reduce(
                red[:, 0:1], u_psum[:], mybir.AxisListType.X, Alu.add
            )
            nc.sync.dma_start(s_dram[:], s_res[:])
            nc.sync.dma_start(r_dram[:], red[:])

    nc.compile()
    return nc


def _get_program():
    if "nc" not in _CACHE:
        _CACHE["nc"] = _build_program()
    return _CACHE["nc"]


def _run(heatmap: np.ndarray, trace: bool = False):
    from concourse.bass_utils import run_bass_kernel_spmd

    nc = _get_program()
    in_maps = [
        {"x": np.ascontiguousarray(heatmap[i].reshape(C, P, F), dtype=np.float32)}
        for i in range(NCORES)
    ]
    return run_bass_kernel_spmd(nc, in_maps, list(range(NCORES)), trace=trace)


def _finalize(results) -> np.ndarray:
    """Host epilogue: a few scalars per core -> entropy[n] in float64."""
    out = np.zeros(N, dtype=np.float64)
    for n in range(NCORES):
        r = results[n]
        s_full = r["s_out"].astype(np.float64).sum(axis=0)   # [SCOLS]
        s_prime = s_full[:C].copy()
        for scol, c in EXTRAS:
            s_prime[c] += s_full[scol]
        red = r["red_out"].astype(np.float64)                # [C, 2]
        u = red[:, 0]
        cnt = red[:, 1]
        s = s_prime - (HW - cnt)                             # masked sum exp
        ent = np.zeros(C, dtype=np.float64)
        ok = s > 0
        ent[ok] = (np.log(s[ok]) - u[ok] / s[ok]) / LN2
        out[n] = ent.sum() / cnt.sum()
    return out.astype(np.float32)


def kernel(heatmap: np.ndarray) -> np.ndarray:
    heatmap = np.asarray(heatmap, dtype=np.float32)
    assert heatmap.shape == (N, C, H, W), heatmap.shape
    res = _run(heatmap, trace=False)
    return _finalize(res.results)
